# revision 40
# baseline (speedup 1.0000x reference)
"""Multi-head self-attention Bass kernel for Trainium2, 8 NeuronCores.

Problem: B=4, S=2048, D=1024, H=16 heads (dh=64), causal mask, fp32.

Sharding: core c -> batch b = c//2, head-group g = c%2 (8 heads each).
Data-parallel over B, tensor-parallel over heads; out-proj is row-parallel
with the partial-sum reduction done on the host (plus all output biases,
which fold into a single per-feature vector added on the host).

Design: the attention inner loop is software-pipelined with a delayed
P@V queue. HW ablations showed the previous layout lost ~130us to the
exp: each P@V matmul sat at the head of the PE's strict-FIFO queue
waiting on its exp, head-of-line-blocking everything behind it. Now,
per (head-pair, k-block) iteration the scores matmuls and the exp are
emitted immediately, but the P@V matmuls are held back `lag`=2
iterations: the PE queue always holds the next iterations' scores plus
proj/out-proj filler ahead of any exp-dependent P@V, so the ~0.65us/
iteration ACT exp stream runs one-to-two iterations ahead and is fully
hidden (removing the exp entirely no longer changes the runtime).
Each pair's softmax normalize rides the same delayed queue as two
phases with no PE involvement (a second head-of-line fix worth ~15us):
the ones column sits FIRST in v_ext so the denominator lands at PSUM
partition 0, where the DVE reciprocal's output row is legal input for
the otherwise-idle Pool engine's partition_broadcast (it can only read
partition 0); a DVE multiply then scales the pair's context and a DMA
partition-shifts it into ctx_sb (consumed a whole q-block later, so
the DMA latency is slack). Chunk c+1's projections and q-block c-1's
out-proj are drained between iterations as PE filler in ~0.9us
half-chain thunks.
All tensors are bf16 except the f32 PSUM accumulators, the biases and
the reciprocal rows; rel err vs the fp32 reference is ~5.8e-3.
HW: ~250-270us (from the 387-395us baseline this session started at).
"""

import sys

sys.path.insert(0, "/opt/trn_rl_repo")

import ml_dtypes
import numpy as np

import concourse.bass as bass  # noqa: F401
import concourse.mybir as mybir
import concourse.tile as tile
from concourse import bacc
from concourse.bass_utils import run_bass_kernel_spmd  # noqa: F401

B = 4
S = 2048
D = 1024
H = 16
DH = 64
N_CORES = 8
HPC = H // 2          # heads per core = 8
FL = HPC * DH         # local feature width = 512
QB = 512              # q-block width
KB = 128              # k-block width
ND = D // 128         # d-blocks = 8
NFB = 8               # qk f-blocks (4 q + 4 k)

F32 = mybir.dt.float32
F32R = mybir.dt.float32r
BF16 = mybir.dt.bfloat16
SCALE = 1.0 / np.sqrt(DH)

_COMPILED = {}


def build_nc(s: int = S, body_reps: int = 1, diag_restrict: bool = True,
             abl: frozenset = frozenset(), lag: int = 2):
    # abl: timing-only ablation switches (never used for correctness runs):
    #   "no_exp"   - skip the exp activations
    #   "no_band"  - skip the causal band multiplies
    #   "no_norm"  - skip softmax normalization (recip/broadcast/mul)
    #   "no_dve"   - skip bias-add/v-copy/out-copy DVE moves
    #   "nopack"   - issue both score matmuls from PE rows 0:64 (defeats
    #                row-group packing; wrong results)
    nqb = s // QB
    nst = s // 128

    nc = bacc.Bacc("TRN2", target_bir_lowering=False, debug=False,
                   num_devices=N_CORES)

    xT = nc.declare_dram_parameter("xT", [D, s], BF16, isOutput=False)
    wqkT = nc.declare_dram_parameter("wqkT", [D, 2 * FL], BF16, isOutput=False)
    wvT = nc.declare_dram_parameter("wvT", [D, FL], BF16, isOutput=False)
    b_qk = nc.declare_dram_parameter("b_qk", [128, NFB], F32, isOutput=False)
    woT = nc.declare_dram_parameter("woT", [FL, D], BF16, isOutput=False)
    band = nc.declare_dram_parameter("band", [nqb * QB, QB], BF16, isOutput=False)
    out = nc.declare_dram_parameter("out", [s, D], BF16, isOutput=True)

    def _touch(sl):
        # keep an ablated tile allocated (pool release asserts otherwise)
        nc.vector.memset(sl, 0.03125)

    with tile.TileContext(nc) as tc:
        with (
            tc.tile_pool(name="persist", bufs=1) as pp,
            tc.tile_pool(name="mmpsum", bufs=2, space="PSUM") as psA,
            tc.tile_pool(name="scpsum", bufs=2, space="PSUM") as psS,
            tc.tile_pool(name="ctxpsum", bufs=2, space="PSUM") as psC,
        ):
            kT_sb = pp.tile([128, HPC // 2, s], BF16)     # k f-tiles
            v_ext = pp.tile([128, nst, HPC, DH + 1], BF16)
            if "no_dve" in abl:
                _touch(kT_sb[:, 0, 0:1])
                _touch(v_ext[:, 0, 0, 0:1])
            woT_sb = pp.tile([128, FL // 128, D], BF16)
            wqk_sb = pp.tile([128, ND, 2 * FL], BF16)
            wvT_sb = pp.tile([128, ND, FL], BF16)
            bqk_sb = pp.tile([128, NFB], F32)
            ones_sb = pp.tile([128, HPC], F32)
            nc.vector.memset(ones_sb[:], 1.0)
            onesf = pp.tile([128, 64], F32)
            nc.vector.memset(onesf[:], 1.0)
            ones64 = pp.tile([128, 64], F32R)
            nc.vector.tensor_copy(ones64[:], onesf[:])

            for rep in range(body_reps):
                with (
                    tc.tile_pool(name=f"px_{rep}", bufs=3) as px,
                    tc.tile_pool(name=f"pq_{rep}", bufs=3) as pq,
                    tc.tile_pool(name=f"pband_{rep}", bufs=3) as pband,
                    tc.tile_pool(name=f"pP_{rep}", bufs=8) as pP,
                    tc.tile_pool(name=f"pN_{rep}", bufs=4) as pN,
                    tc.tile_pool(name=f"pctx_{rep}", bufs=3) as pctx,
                    tc.tile_pool(name=f"pout_{rep}", bufs=4) as pout,
                ):
                    # ---------- DMA prefetches, priority order ----------
                    wqkv = wqkT.ap().rearrange("(a p) f -> p a f", p=128)
                    xv = xT.ap().rearrange("(a p) s -> p a s", p=128)
                    xt_tiles = []
                    xt0 = px.tile([128, ND, QB], BF16, tag="xT")
                    xt_tiles.append(xt0)
                    # interleave the first k-weight block with chunk-0 x so
                    # the first projection group starts as early as possible
                    def _wqk_dma(fb):
                        nc.sync.dma_start(
                            wqk_sb[:, :, 128 * fb:128 * (fb + 1)],
                            wqkv[:, :, 128 * fb:128 * (fb + 1)])
                    # priority: the pair-0 scores chain (k f-tile 0 needs
                    # fb4, q f-tile 0 needs fb0) so the first exp can start
                    # as early as the DMA stream allows
                    _wqk_dma(4)
                    for d in range(4):
                        nc.sync.dma_start(xt0[:, d, :], xv[:, d, 0:QB])
                    _wqk_dma(0)
                    for d in range(4, ND):
                        nc.sync.dma_start(xt0[:, d, :], xv[:, d, 0:QB])
                    nc.sync.dma_start(bqk_sb[:], b_qk.ap())
                    for fb in (5, 1, 6, 2, 7, 3):
                        _wqk_dma(fb)
                    nc.sync.dma_start(
                        wvT_sb[:], wvT.ap().rearrange("(a p) f -> p a f", p=128))
                    # chunk 1 x
                    xt1 = px.tile([128, ND, QB], BF16, tag="xT")
                    xt_tiles.append(xt1)
                    for d in range(ND):
                        nc.sync.dma_start(xt1[:, d, :], xv[:, d, QB:2 * QB])
                    bandv = band.ap().rearrange("(i k p) q -> p (i k) q",
                                                p=128, k=4)
                    band_tiles = {}
                    band_tiles[0] = pband.tile([128, 4, QB], BF16, tag="band", name="band0")
                    nc.sync.dma_start(band_tiles[0][:], bandv[:, 0:4, :])
                    nc.sync.dma_start(
                        woT_sb[:], woT.ap().rearrange("(a p) d -> p a d", p=128))

                    if rep == 0:
                        # p-state warmup: dead matmuls on resident data while
                        # the first DMAs stream; ramps the PE clock so real
                        # work starts at full speed. Results are never read.
                        for _ in range(20):
                            pw = psA.tile([128, QB], F32, tag="mm")
                            nc.tensor.matmul(pw[0:64, 0:64],
                                             ones64[64:65, 0:64],
                                             ones64[64:65, 0:64],
                                             start=True, stop=True)

                    # remaining chunk x tiles are allocated lazily inside the
                    # loop (pool bufs=2 recycles); their DMAs are emitted when
                    # allocated, which is early enough given the interleave.

                    # ---------- work-item generators ----------
                    def proj_chunk_items(c, xt, qt):
                        """Yield half-chain thunks (~0.9us PE each) emitting
                        chunk c's projections. Order: k/q f-tiles, v tiles."""
                        sl = slice(QB * c, QB * (c + 1))

                        def mk_kq(fb, dest, dcol):
                            box = {}

                            def emitA():
                                ps = psA.tile([128, QB], F32, tag="mm")
                                box["ps"] = ps
                                for d in range(4):
                                    nc.tensor.matmul(
                                        ps[:],
                                        wqk_sb[:, d, 128 * fb:128 * (fb + 1)],
                                        xt[:, d, :],
                                        start=(d == 0), stop=False)

                            def emitB():
                                ps = box["ps"]
                                for d in range(4, ND):
                                    nc.tensor.matmul(
                                        ps[:],
                                        wqk_sb[:, d, 128 * fb:128 * (fb + 1)],
                                        xt[:, d, :],
                                        start=False, stop=(d == ND - 1))
                                if "no_dve" not in abl:
                                    nc.vector.tensor_scalar_add(
                                        dest[:, dcol, sl] if dest is kT_sb
                                        else dest[:, dcol, :],
                                        ps[:], bqk_sb[:, fb:fb + 1])
                            return [emitA, emitB]

                        def mk_v(st4):
                            st = 4 * c + st4
                            box = {}

                            def emitA():
                                ps = psA.tile([128, FL], F32, tag="mm")
                                box["ps"] = ps
                                for d in range(4):
                                    nc.tensor.matmul(
                                        ps[:],
                                        xt[:, d, 128 * st4:128 * (st4 + 1)],
                                        wvT_sb[:, d, :],
                                        start=(d == 0), stop=False)

                            def emitB():
                                ps = box["ps"]
                                for d in range(4, ND):
                                    nc.tensor.matmul(
                                        ps[:],
                                        xt[:, d, 128 * st4:128 * (st4 + 1)],
                                        wvT_sb[:, d, :],
                                        start=False, stop=(d == ND - 1))
                                if "no_dve" not in abl:
                                    nc.vector.tensor_copy(
                                        v_ext[:, st, :, 1:DH + 1],
                                        ps[:].rearrange("p (h e) -> p h e",
                                                        h=HPC))
                                    nc.vector.tensor_copy(
                                        v_ext[:, st, :, 0], ones_sb[:])
                            return [emitA, emitB]

                        # k then q then v: the NEXT q-block's first scores
                        # need k and q early, while its PV only reaches this
                        # chunk's v tiles at the end of its kb range.
                        for hp in range(4):        # k tiles (combined fb 4+hp)
                            yield from mk_kq(4 + hp, kT_sb, hp)
                        for hp in range(4):        # q tiles
                            yield from mk_kq(hp, qt, hp)
                        for st4 in range(4):       # v tiles
                            yield from mk_v(st4)

                    def outproj_items(qb, ctx_sb):
                        def mk(st4):
                            box = {}

                            def half(dh2):
                                def emit():
                                    if dh2 == 0:
                                        box["ot"] = pout.tile([128, D], BF16,
                                                              tag="ot",
                                                              name="ot")
                                        if "no_dve" in abl:
                                            _touch(box["ot"][:, 0:1])
                                    ot = box["ot"]
                                    po = psA.tile([128, 512], F32, tag="mm")
                                    for fb2 in range(FL // 128):
                                        nc.tensor.matmul(
                                            po[:],
                                            ctx_sb[:, fb2,
                                                   128 * st4:128 * (st4 + 1)],
                                            woT_sb[:, fb2,
                                                   512 * dh2:512 * (dh2 + 1)],
                                            start=(fb2 == 0), stop=(fb2 == 3))
                                    if "no_dve" not in abl:
                                        if "actout" in abl:
                                            nc.scalar.copy(
                                                ot[:,
                                                   512 * dh2:512 * (dh2 + 1)],
                                                po[:])
                                        else:
                                            nc.vector.tensor_copy(
                                                ot[:,
                                                   512 * dh2:512 * (dh2 + 1)],
                                                po[:])
                                    if dh2 == 1:
                                        st = 4 * qb + st4
                                        nc.sync.dma_start(
                                            out[128 * st:128 * (st + 1), :],
                                            ot[:])
                                return emit
                            return [half(0), half(1)]
                        for st4 in range(QB // 128):
                            yield from mk(st4)

                    # ---- softmax normalize, two delayed phases ----
                    # phase A: DVE reciprocals of the ones-column denominators
                    # phase B: gpsimd partition-broadcast of the recip row,
                    #          then DVE multiplies into ctx (h1 via DMA
                    #          partition shift).  No PE work at all.
                    norm_recs = {}

                    def mk_normA(hp, pc0, pc1):
                        def em():
                            if "no_norm" in abl:
                                return
                            # evict both pc banks to SBUF immediately (one
                            # copy on ACT, one on DVE, in parallel): the
                            # next pair's first P@V only waits these two
                            # copies, not the whole normalize chain
                            evs, recs = [], []
                            for j2, pc in enumerate((pc0, pc1)):
                                sc = pN.tile([DH + 1, QB], F32, tag="scc",
                                             name="scc")
                                if j2 == 0:
                                    nc.scalar.copy(sc[:], pc[0:DH + 1, :])
                                else:
                                    nc.vector.tensor_copy(sc[:],
                                                          pc[0:DH + 1, :])
                                evs.append(sc)
                            for sc in evs:
                                rec = pN.tile([1, QB], F32, tag="rec")
                                # ones column sits FIRST in v_ext, so the
                                # denominator is row 0 - the only partition
                                # gpsimd partition_broadcast can read from
                                nc.vector.reciprocal(rec[0:1, :],
                                                     sc[0:1, :])
                                recs.append(rec)
                            norm_recs[hp] = (evs, recs)
                        return em

                    def mk_normB(hp, pc0, pc1, ctx_sb):
                        def em():
                            if "no_norm" in abl:
                                return
                            evs, recs = norm_recs.pop(hp)
                            for j2, (sc, rec) in enumerate(zip(evs, recs)):
                                bc = pN.tile([65, QB], F32, tag="bc")
                                nc.gpsimd.partition_broadcast(bc[:],
                                                              rec[0:1, :])
                                tmp = pN.tile([128, QB], BF16, tag="tmp")
                                # DVE partition base must be 32-aligned:
                                # include row 0 (d * 1/d, unused) in the mul
                                nc.vector.tensor_mul(tmp[0:DH + 1, :],
                                                     sc[:],
                                                     bc[0:DH + 1, :])
                                # partition shift into ctx (DMA; consumed an
                                # entire q-block later, so latency is slack)
                                nc.sync.dma_start(
                                    ctx_sb[64 * j2:64 * (j2 + 1), hp, :],
                                    tmp[1:DH + 1, :])
                        return em

                    def attention_emit(qb, qt, band_sb, ctx_sb, filler):
                        """Software-pipelined attention for q-block qb.

                        Per (pair, kb) iteration the scores matmuls and the
                        exp are emitted immediately, but the P@V matmuls are
                        held in a delayed queue and emitted `lag` iterations
                        later: the PE then always has the next iterations'
                        scores plus filler in its queue ahead of a PV that
                        waits on an exp, so the ACT exp stream is covered
                        instead of head-of-line-blocking the PE.  Each pair's
                        normalize rides the same queue as two phases (recip,
                        broadcast+mul) right after the pair's last PV."""
                        nkb = 4 * (qb + 1)
                        niter = 4 * nkb
                        fq = list(filler)
                        fi = 0
                        fill_budget = 0.0
                        per_iter = len(fq) / max(niter, 1)
                        dq = []      # (ready_iter, thunk), FIFO by append
                        it = 0

                        def mk_pv(pc0, pc1, pt, kb, off, h0, h1, first, last):
                            def em():
                                nc.tensor.matmul(
                                    pc0[:, off:], v_ext[:, kb, h0, :],
                                    pt[:, 0, off:], start=first, stop=last)
                                nc.tensor.matmul(
                                    pc1[:, off:], v_ext[:, kb, h1, :],
                                    pt[:, 1, off:], start=first, stop=last)
                            return em

                        for hp in range(HPC // 2):
                            h0, h1 = 2 * hp, 2 * hp + 1
                            qT0 = qt[0:64, hp, :]
                            qT1 = qt[64:128, hp, :]
                            pc0 = psC.tile([DH + 1, QB], F32, tag="ctxp")
                            pc1 = psC.tile([DH + 1, QB], F32, tag="ctxp")
                            for kb in range(nkb):
                                diag = kb >= 4 * qb
                                j = kb - 4 * qb
                                off = 128 * j if (diag and diag_restrict) else 0
                                ps = psS.tile([128, 2, QB], F32, tag="sc")
                                kcol = slice(128 * kb, 128 * (kb + 1))
                                nc.tensor.matmul(
                                    ps[:, 0, off:], kT_sb[0:64, hp, kcol],
                                    qT0[:, off:], start=True, stop=True)
                                if "nopack" in abl:
                                    nc.tensor.matmul(
                                        ps[:, 1, off:], kT_sb[0:64, hp, kcol],
                                        qT0[:, off:], start=True, stop=True)
                                else:
                                    nc.tensor.matmul(
                                        ps[:, 1, off:], kT_sb[64:128, hp, kcol],
                                        qT1[:, off:], start=True, stop=True)
                                pt = pP.tile([128, 2, QB], BF16, tag="P")
                                if "no_exp" not in abl:
                                    nc.scalar.activation(
                                        pt[:, :, off:], ps[:, :, off:],
                                        mybir.ActivationFunctionType.Exp,
                                        scale=float(SCALE))
                                elif "no_dve" in abl:
                                    _touch(pt[:, 0, 0:1])
                                else:
                                    # keep dependency structure: tiny exp
                                    nc.scalar.activation(
                                        pt[:, :, 0:1], ps[:, :, 0:1],
                                        mybir.ActivationFunctionType.Exp,
                                        scale=float(SCALE))
                                if diag and "no_band" not in abl:
                                    mo = 128 * j
                                    lo = off if diag_restrict else 0
                                    beng = (nc.gpsimd if "gpband" in abl
                                            else nc.vector)
                                    beng.tensor_mul(
                                        pt[:, :, lo:mo + 128],
                                        pt[:, :, lo:mo + 128],
                                        band_sb[:, j, None, lo:mo + 128]
                                        .to_broadcast((128, 2, mo + 128 - lo)))
                                dq.append((it + lag, mk_pv(
                                    pc0, pc1, pt, kb, off, h0, h1,
                                    kb == 0, kb == nkb - 1)))
                                if kb == nkb - 1:
                                    dq.append((it + lag,
                                               mk_normA(hp, pc0, pc1)))
                                    dq.append((it + lag,
                                               mk_normB(hp, pc0, pc1,
                                                        ctx_sb)))
                                while dq and dq[0][0] <= it:
                                    dq.pop(0)[1]()
                                it += 1
                                fill_budget += per_iter
                                while fi < len(fq) and fill_budget >= 1.0:
                                    fq[fi]()
                                    fi += 1
                                    fill_budget -= 1.0
                        # flush: remaining PVs + final normalize, interleaving
                        # any leftover filler as PE cover
                        while dq:
                            dq.pop(0)[1]()
                            if fi < len(fq):
                                fq[fi]()
                                fi += 1
                        while fi < len(fq):
                            fq[fi]()
                            fi += 1

                    # ---------- pipeline ----------
                    qt_tiles = {}
                    ctx_tiles = {}
                    # chunk 0 projections, unfilled
                    qt_tiles[0] = pq.tile([128, 4, QB], BF16, tag="qT", name="qt0")
                    if "no_dve" in abl:
                        _touch(qt_tiles[0][:, 0, 0:1])
                    # k/q interleaved (pair-p scores become ready pair by
                    # pair), v last: the scheduler hoists qb0's first scores
                    # and exp ahead of the v-projections still waiting on
                    # their weight DMA.
                    items0 = list(proj_chunk_items(0, xt_tiles[0],
                                                   qt_tiles[0]))
                    # pairs of half-thunks: k0,q0,k1,q1,... then v tiles
                    for i in (0, 1, 8, 9, 2, 3, 10, 11, 4, 5, 12, 13,
                              6, 7, 14, 15, 16, 17, 18, 19, 20, 21, 22, 23):
                        items0[i]()

                    for qb in range(nqb):
                        filler = []
                        if qb + 1 < nqb:
                            # allocate next chunk's tiles + DMAs now
                            if qb + 1 >= len(xt_tiles):
                                xtn = px.tile([128, ND, QB], BF16, tag="xT")
                                xt_tiles.append(xtn)
                                for d in range(ND):
                                    nc.sync.dma_start(
                                        xtn[:, d, :],
                                        xv[:, d, QB * (qb + 1):QB * (qb + 2)])
                            if qb + 1 not in band_tiles:
                                bt = pband.tile([128, 4, QB], BF16, tag="band", name="bandn")
                                band_tiles[qb + 1] = bt
                                nc.sync.dma_start(
                                    bt[:], bandv[:, 4 * (qb + 1):4 * (qb + 2), :])
                            qt_tiles[qb + 1] = pq.tile([128, 4, QB], BF16, tag="qT", name="qtn")
                            if "no_dve" in abl:
                                _touch(qt_tiles[qb + 1][:, 0, 0:1])
                            filler.extend(proj_chunk_items(
                                qb + 1, xt_tiles[qb + 1], qt_tiles[qb + 1]))
                        if qb - 1 >= 0:
                            filler.extend(outproj_items(qb - 1,
                                                        ctx_tiles[qb - 1]))
                        ctx_tiles[qb] = pctx.tile([128, FL // 128, QB], BF16, tag="ctx", name="ctxq")
                        if abl & {"no_norm", "no_dve"}:
                            _touch(ctx_tiles[qb][:, 0, 0:1])
                        attention_emit(qb, qt_tiles[qb], band_tiles[qb],
                                       ctx_tiles[qb], filler)
                    # last q-block's out-proj (its ctx finishes in the flush)
                    for th in outproj_items(nqb - 1, ctx_tiles[nqb - 1]):
                        th()
    nc.finalize()
    return nc


def prep_in_maps(x, mask, w_qkv, b_qkv, w_out, s: int = S):
    nqb = s // QB
    m = np.asarray(mask)[0, 0]
    band = np.empty((nqb * QB, QB), ml_dtypes.bfloat16)
    for i in range(nqb):
        band[QB * i:QB * (i + 1)] = (~m[QB * i:QB * (i + 1),
                                        QB * i:QB * (i + 1)]).T
    in_maps = []
    for c in range(N_CORES):
        b, g = c // 2, c % 2
        wq = w_qkv[FL * g:FL * (g + 1)]
        wk = w_qkv[D + FL * g:D + FL * (g + 1)]
        wv = w_qkv[2 * D + FL * g:2 * D + FL * (g + 1)]
        bq = b_qkv[FL * g:FL * (g + 1)]
        bk = b_qkv[D + FL * g:D + FL * (g + 1)]
        bqk = np.concatenate([bq, bk]).reshape(2 * FL // 128, 128).T
        in_maps.append({
            "xT": np.ascontiguousarray(x[b].T).astype(ml_dtypes.bfloat16),
            "wqkT": np.ascontiguousarray(
                np.concatenate([wq, wk], 0).T).astype(ml_dtypes.bfloat16),
            "wvT": np.ascontiguousarray(wv.T).astype(ml_dtypes.bfloat16),
            "b_qk": np.ascontiguousarray(bqk),
            "woT": np.ascontiguousarray(
                w_out[:, FL * g:FL * (g + 1)].T).astype(ml_dtypes.bfloat16),
            "band": band,
        })
    return in_maps


def assemble(results, b_qkv, w_out, b_out, s: int = S):
    bv = b_qkv[2 * D:]
    bias_full = b_out + w_out @ bv
    outp = np.empty((B, s, D), np.float32)
    for b in range(B):
        outp[b] = (results[2 * b]["out"].astype(np.float32)
                   + results[2 * b + 1]["out"].astype(np.float32)
                   + bias_full[None, :])
    return outp


def _make_runner(nc, in_maps):
    """Persistent PJRT runner: trace/compile once, reuse the executable."""
    import jax
    from jax.sharding import Mesh, PartitionSpec, NamedSharding
    from jax.experimental.shard_map import shard_map
    from concourse import bass2jax
    from concourse.bass2jax import _bass_exec_p, partition_id_tensor

    bass2jax.install_neuronx_cc_hook()
    partition_name = nc.partition_id_tensor.name if nc.partition_id_tensor else None
    in_names, out_names, out_avals, zero_outs = [], [], [], []
    for alloc in nc.m.functions[0].allocations:
        if not isinstance(alloc, mybir.MemoryLocationSet):
            continue
        name = alloc.memorylocations[0].name
        if alloc.kind == "ExternalInput":
            if name != partition_name:
                in_names.append(name)
        elif alloc.kind == "ExternalOutput":
            out_names.append(name)
            shape = tuple(alloc.tensor_shape)
            dtype = mybir.dt.np(alloc.dtype)
            out_avals.append(jax.core.ShapedArray(shape, dtype))
            zero_outs.append(np.zeros(shape, dtype))
    n_params = len(in_names)
    all_in_names = list(in_names) + list(out_names)
    if partition_name is not None:
        all_in_names.append(partition_name)

    def _body(*args):
        operands = list(args)
        if partition_name is not None:
            operands.append(partition_id_tensor())
        return tuple(_bass_exec_p.bind(
            *operands,
            out_avals=tuple(out_avals),
            in_names=tuple(all_in_names),
            out_names=tuple(out_names),
            lowering_input_output_aliases=(),
            sim_require_finite=True,
            sim_require_nnan=True,
            nc=nc,
        ))

    devices = jax.devices()[:N_CORES]
    mesh = Mesh(np.asarray(devices), ("core",))
    n_out = len(out_names)
    sharded = jax.jit(
        shard_map(_body, mesh=mesh,
                  in_specs=(PartitionSpec("core"),) * (n_params + n_out),
                  out_specs=(PartitionSpec("core"),) * n_out,
                  check_rep=False),
        keep_unused=True,
    )
    sh = NamedSharding(mesh, PartitionSpec("core"))
    concat_zeros = [
        np.zeros((N_CORES * z.shape[0], *z.shape[1:]), z.dtype) for z in zero_outs
    ]
    dev_zeros = [jax.device_put(a, sh) for a in concat_zeros]

    def run(in_maps):
        concat_in = [
            np.concatenate([np.asarray(in_maps[c][nm]) for c in range(N_CORES)], 0)
            for nm in in_names
        ]
        dev_in = [jax.device_put(a, sh) for a in concat_in]
        out = sharded(*dev_in, *dev_zeros)
        jax.block_until_ready(out)
        return [
            {nm: np.asarray(out[i]).reshape(N_CORES, *out_avals[i].shape)[c]
             for i, nm in enumerate(out_names)}
            for c in range(N_CORES)
        ]

    return run


def kernel(x, mask, w_qkv, b_qkv, w_out, b_out):
    x = np.asarray(x, np.float32)
    w_qkv = np.asarray(w_qkv, np.float32)
    b_qkv = np.asarray(b_qkv, np.float32)
    w_out = np.asarray(w_out, np.float32)
    b_out = np.asarray(b_out, np.float32)

    in_maps = prep_in_maps(x, mask, w_qkv, b_qkv, w_out)
    if "run" not in _COMPILED:
        _COMPILED["nc"] = build_nc()
        _COMPILED["run"] = _make_runner(_COMPILED["nc"], in_maps)
    results = _COMPILED["run"](in_maps)
    return assemble(results, b_qkv, w_out, b_out)

